# revision 68
# baseline (speedup 1.0000x reference)
"""Trainium2 Bass kernel for a single-step decoder BiRNN with attention + vocab projection.

Computation (see reference):
  x = emb[tok]                                  (1024,)
  h_d = tanh(W_ih[d] x + b_ih[d] + W_hh[d] h0_d + b_hh[d])        d in {0,1}
  energy = h @ enc^T -> softmax over L -> attn_out = attn @ enc   (2, 1024)
  u = concat(attn_out0, h0, attn_out1, h1)      (4096,)
  logits = W_out @ u + b_out                    (50257,)
  out = log_softmax(logits), also return h

Sharding: W_out rows (vocab) split across 8 cores (6400 padded rows each,
pre-transposed on the host so DMA lines are contiguous).  The tiny RNN cell and
attention are replicated on every core.  Each core returns its logits shard plus
per-slab (max, sum-exp) partials; the host combines those into the global
log-softmax normalizer.
"""

import os
import sys
from contextlib import ExitStack

import numpy as np

if "/opt/trn_rl_repo" not in sys.path:
    sys.path.insert(0, "/opt/trn_rl_repo")

import concourse.bass as bass
import concourse.tile as tile
from concourse import bacc, mybir
from concourse.bass_utils import run_bass_kernel_spmd

H = 1024
L = 2048
V = 50257
NCORES = 8
NV = 6400            # per-core padded vocab rows (8 * 6400 = 51200 >= V)
K = 4 * H            # contraction dim of the output projection
NEG = -1e9           # bias for padded vocab rows (exp underflows to exactly 0)

F32 = mybir.dt.float32
F32R = mybir.dt.float32r   # full-rate PE at N>=256, same bits as fp32
BF16 = mybir.dt.bfloat16
AX = mybir.AxisListType
AF = mybir.ActivationFunctionType
ALU = mybir.AluOpType

# (voff, vlen, nslabs): sequential passes over the vocab shard so the live
# [1, 512] PSUM accumulators never exceed the 8 banks.
PASSES = [(0, 2048, 4), (2048, 2048, 4), (4096, 2048, 4), (6144, 256, 1)]
NSLABS = 13

# Results of the last device run (test harness reads exec_time_ns from here).
LAST_RESULTS = None


def _build_body(ctx: ExitStack, tc: tile.TileContext, ins, outs):
    stage = int(os.environ.get("KSTAGE", "4"))  # debug bisect knob
    nc = tc.nc
    xh, wcat_t, bias_h, enc_t, enc_n, id2, w_out_t, bias_v = ins
    logits_o, stats_o, h_o = outs

    const = ctx.enter_context(tc.tile_pool(name="const", bufs=1))
    small = ctx.enter_context(tc.tile_pool(name="small", bufs=2))
    wcat_pool = ctx.enter_context(tc.tile_pool(name="wcat", bufs=2))
    w_pool = ctx.enter_context(tc.tile_pool(name="w", bufs=6))
    scr_pool = ctx.enter_context(tc.tile_pool(name="scr", bufs=2))
    encn_pool = ctx.enter_context(tc.tile_pool(name="encn", bufs=2))
    bvs_pool = ctx.enter_context(tc.tile_pool(name="bvs", bufs=2))
    lgsb_pool = ctx.enter_context(tc.tile_pool(name="lgsb", bufs=2))
    ps_pool = ctx.enter_context(tc.tile_pool(name="ps", bufs=4, space="PSUM"))

    # ---------------- small resident inputs ----------------
    xh_sb = const.tile([128, 16, 2], F32R)
    nc.sync.dma_start(xh_sb[:], xh[:])
    biash_sb = const.tile([1, 2 * H], F32)
    nc.sync.dma_start(biash_sb[:], bias_h[:])
    one_sb = const.tile([1, 1], F32)
    nc.vector.memset(one_sb[:], 1.0)
    id2_sb = const.tile([2, 2], F32)
    nc.sync.dma_start(id2_sb[:], id2[:])

    # u vector as 32 column chunks: col j = 16*d + 8*s + c  (s=0: attn_out, s=1: h)
    u_sb = const.tile([128, 32], F32R)
    u4 = u_sb.rearrange("p (d s c) -> p d s c", d=2, s=2)

    # ---------------- RNN cell: h_d = tanh(Wcat_d @ ucat_d + b_d) ----------------
    h_pre = [[ps_pool.tile([1, 512], F32, name=f"hpre_{d}_{hs}", tag="pbank", bufs=4) for hs in range(2)] for d in range(2)]
    for d in range(2):
        wc_r = wcat_t.rearrange("d (c p) h -> p d c h", p=128)
        for ci in range(8):  # two 128-row k-chunks per 1MB DMA
            wt = wcat_pool.tile([128, 2, H], F32R, tag="wcat")
            nc.sync.dma_start(wt[:], wc_r[:, d, 2 * ci : 2 * ci + 2, :])
            for cc in range(2):
                c = 2 * ci + cc
                for hs in range(2):
                    nc.tensor.matmul(
                        h_pre[d][hs][0:1, :],
                        lhsT=xh_sb[:, c : c + 1, d],
                        rhs=wt[:, cc, 512 * hs : 512 * hs + 512],
                        start=(c == 0),
                        stop=(c == 15),
                    )

    # enc^T resident: [p, hc, l] = enc_t[128*hc + p, l].  Emitted after the
    # recurrent-weight stream so wcat keeps DMA priority (the whole attention
    # chain is gated on it); split into 4 transfers for queue balance.
    enc_sb = const.tile([128, 8, L], F32R)
    enc_r = enc_t.rearrange("(hc p) l -> p hc l", p=128)
    for g in range(4):
        nc.sync.dma_start(enc_sb[:, 2 * g : 2 * g + 2, :], enc_r[:, 2 * g : 2 * g + 2, :])

    h_sb = const.tile([1, 2 * H], F32)
    for d in range(2):
        for hs in range(2):
            lo = d * H + hs * 512
            tmp = small.tile([1, 512], F32, tag="htmp")
            nc.vector.tensor_add(tmp[:], h_pre[d][hs][0:1, :], biash_sb[0:1, lo : lo + 512])
            nc.scalar.activation(h_sb[0:1, lo : lo + 512], tmp[:], AF.Tanh)

    # transpose h into u columns via K=1 outer product with ones
    for d in range(2):
        tp = ps_pool.tile([128, 8], F32, name=f"tp_{d}", tag="pbank", bufs=4)
        for c in range(8):
            nc.tensor.matmul(
                tp[:, c : c + 1],
                lhsT=h_sb[0:1, (d * 8 + c) * 128 : (d * 8 + c) * 128 + 128],
                rhs=one_sb[0:1, 0:1],
                start=True,
                stop=True,
            )
        nc.vector.tensor_copy(u4[:, d, 1, :], tp[:])

    nc.sync.dma_start(h_o.rearrange("p (d c) -> p d c", d=2), u4[:, :, 1, :])
    if stage < 2:
        return

    # bf16 u for the projection; h columns are ready now, attn columns later.
    # Per-direction copies so part 1 can start on d=0's chunks while d=1's
    # recurrent weights are still streaming.
    u_bf = const.tile([128, 32], BF16)
    ub4 = u_bf.rearrange("p (d s c) -> p d s c", d=2, s=2)
    for d in range(2):
        nc.vector.tensor_copy(ub4[:, d, 1, :], u4[:, d, 1, :])

    # partial logits accumulator (h-chunk contribution, computed pre-attention
    # so the W_out stream never stalls on the attention tail)
    part_sb = const.tile([1, NV], F32)
    w_r = w_out_t.rearrange("(c p) v -> p c v", p=128)
    mv = const.tile([1, 16], F32)
    negmv = const.tile([1, 16], F32)
    sv = const.tile([1, 16], F32)
    nc.vector.memset(mv[:], 0.0)
    nc.vector.memset(negmv[:], 0.0)
    nc.vector.memset(sv[:], 0.0)

    def projection_part(pair_starts, store_partial):
        idx = 0
        tagc = "p" if store_partial else "f"
        for voff, vlen, nslabs in PASSES:
            psums = [
                ps_pool.tile(
                    [1, 512], F32, name=f"lg{tagc}_{voff}_{s}", tag="pbank", bufs=4
                )
                for s in range(nslabs)
            ]
            for pi, c0 in enumerate(pair_starts):
                wt = w_pool.tile([128, 2, 2048], BF16, tag="w")
                nc.sync.dma_start(
                    wt[:, :, :vlen], w_r[:, c0 : c0 + 2, voff : voff + vlen]
                )
                for cc in range(2):
                    for s in range(nslabs):
                        so = 512 * s
                        sl = min(512, vlen - so)
                        nc.tensor.matmul(
                            psums[s][0:1, :sl],
                            lhsT=u_bf[:, c0 + cc : c0 + cc + 1],
                            rhs=wt[:, cc, so : so + sl],
                            start=(pi == 0 and cc == 0),
                            stop=(pi == len(pair_starts) - 1 and cc == 1),
                        )
            for s in range(nslabs):
                so = 512 * s
                sl = min(512, vlen - so)
                g0 = voff + so
                if store_partial:
                    nc.vector.tensor_copy(part_sb[0:1, g0 : g0 + sl], psums[s][0:1, :sl])
                else:
                    bvt = bvs_pool.tile([1, 512], F32, name=f"bv_{g0}", tag="bvs")
                    nc.sync.dma_start(bvt[0:1, :sl], bias_v[0:1, g0 : g0 + sl])
                    lgt = lgsb_pool.tile([1, 512], F32, name=f"lgsb_{g0}", tag="lgsb")
                    nc.vector.tensor_add(
                        lgt[0:1, :sl], psums[s][0:1, :sl], part_sb[0:1, g0 : g0 + sl]
                    )
                    nc.vector.tensor_add(lgt[0:1, :sl], lgt[0:1, :sl], bvt[0:1, :sl])
                    nc.vector.reduce_max(
                        mv[0:1, idx : idx + 1], lgt[0:1, :sl], axis=AX.X
                    )
                    nc.scalar.mul(
                        negmv[0:1, idx : idx + 1], mv[0:1, idx : idx + 1], -1.0
                    )
                    scr = scr_pool.tile([1, 512], F32, tag="exps")
                    nc.scalar.activation(
                        scr[0:1, :sl],
                        lgt[0:1, :sl],
                        AF.Exp,
                        bias=negmv[0:1, idx : idx + 1],
                        accum_out=sv[0:1, idx : idx + 1],
                    )
                    nc.sync.dma_start(logits_o[0:1, g0 : g0 + sl], lgt[0:1, :sl])
                idx += 1

    # part 1: h-chunk columns of W_out (u columns 8..15 and 24..31)
    projection_part([8, 10, 12, 14, 24, 26, 28, 30], store_partial=True)

    # ---------------- attention ----------------
    # energy[d, l] = sum_h h[d, h] * enc[l, h] for both directions at once
    # (fp32 for accuracy: the softmax is sensitive to energy error).
    eps2 = ps_pool.tile([2, L], F32, name="eps2", tag="pwide", bufs=1)
    for hc in range(8):
        for lt in range(4):
            nc.tensor.matmul(
                eps2[:, 512 * lt : 512 * lt + 512],
                lhsT=u4[:, :, 1, hc],
                rhs=enc_sb[:, hc, 512 * lt : 512 * lt + 512],
                start=(hc == 0),
                stop=(hc == 7),
            )

    # joint softmax over the free (L) axis for both directions
    st = const.tile([2, 4], F32)  # cols: m, -m, s, 1/s
    attn2 = const.tile([2, L], F32)
    nc.vector.reduce_max(st[:, 0:1], eps2[:], axis=AX.X)
    nc.scalar.mul(st[:, 1:2], st[:, 0:1], -1.0)
    nc.scalar.activation(
        attn2[:], eps2[:], AF.Exp, bias=st[:, 1:2], accum_out=st[:, 2:3]
    )
    nc.vector.reciprocal(st[:, 3:4], st[:, 2:3])
    nc.vector.tensor_scalar_mul(attn2[:], attn2[:], st[:, 3:4])

    if stage < 3:
        return
    # ---------------- attn_out = attn @ enc (PE path, bf16) ----------------
    # transpose attn into l-partition chunks via identity: attnT_sb[p, lc, d]
    attnT_sb = const.tile([128, 16, 2], BF16)
    for g in range(2):
        tpa = ps_pool.tile([128, 16], F32, name=f"tpa_{g}", tag="pwide", bufs=1)
        for j in range(8):
            lc = g * 8 + j
            nc.tensor.matmul(
                tpa[:, 2 * j : 2 * j + 2],
                lhsT=attn2[:, 128 * lc : 128 * lc + 128],
                rhs=id2_sb[:],
                start=True,
                stop=True,
            )
        nc.vector.tensor_copy(
            attnT_sb[:, 8 * g : 8 * g + 8, :], tpa[:].rearrange("p (c d) -> p c d", d=2)
        )

    # attn_out[d, :] = sum_l attn[d, l] * enc_n[l, :] (bf16, both directions)
    ao_all = ps_pool.tile([2, 2, 512], F32, name="ao_all", tag="pwide", bufs=1)
    ao_ps = [ao_all[:, hs, :] for hs in range(2)]
    en_r = enc_n.rearrange("(c p) h -> p c h", p=128)
    for lci in range(8):
        et = encn_pool.tile([128, 2, H], BF16, tag="encn")
        nc.sync.dma_start(et[:], en_r[:, 2 * lci : 2 * lci + 2, :])
        for j in range(2):
            lc = 2 * lci + j
            for hs in range(2):
                nc.tensor.matmul(
                    ao_ps[hs],
                    lhsT=attnT_sb[:, lc, :],
                    rhs=et[:, j, 512 * hs : 512 * hs + 512],
                    start=(lc == 0),
                    stop=(lc == 15),
                )

    # transpose attn_out into u columns via identity
    ao_sb = const.tile([2, H], F32)
    for hs in range(2):
        nc.vector.tensor_copy(ao_sb[:, 512 * hs : 512 * hs + 512], ao_ps[hs])
    tpo = ps_pool.tile([128, 16], F32, name="tpo", tag="pwide", bufs=1)
    for c in range(8):
        nc.tensor.matmul(
            tpo[:, 2 * c : 2 * c + 2],
            lhsT=ao_sb[:, 128 * c : 128 * c + 128],
            rhs=id2_sb[:],
            start=True,
            stop=True,
        )
    nc.vector.tensor_copy(u4[:, :, 0, :], tpo[:].rearrange("p (c d) -> p d c", d=2))
    nc.vector.tensor_copy(ub4[:, :, 0, :], tpo[:].rearrange("p (c d) -> p d c", d=2))

    if stage < 4:
        return
    # part 2: attn-chunk columns of W_out (u columns 0..7 and 16..23)
    projection_part([0, 2, 4, 6, 16, 18, 20, 22], store_partial=False)

    # ---------------- outputs ----------------
    nc.sync.dma_start(stats_o[0:1, :], mv[:])
    nc.sync.dma_start(stats_o[1:2, :], sv[:])


_CACHED_NC = None


def _get_program():
    global _CACHED_NC
    if _CACHED_NC is not None:
        return _CACHED_NC
    nc = bacc.Bacc(
        "TRN2",
        target_bir_lowering=False,
        debug=False,
        enable_asserts=False,
        num_devices=NCORES,
    )
    ins = [
        nc.dram_tensor("xh", [128, 16, 2], F32R, kind="ExternalInput").ap(),
        nc.dram_tensor("wcat_t", [2, 2 * H, H], F32R, kind="ExternalInput").ap(),
        nc.dram_tensor("bias_h", [1, 2 * H], F32, kind="ExternalInput").ap(),
        nc.dram_tensor("enc_t", [H, L], F32R, kind="ExternalInput").ap(),
        nc.dram_tensor("enc_n", [L, H], BF16, kind="ExternalInput").ap(),
        nc.dram_tensor("id2", [2, 2], F32, kind="ExternalInput").ap(),
        nc.dram_tensor("w_out_t", [K, NV], BF16, kind="ExternalInput").ap(),
        nc.dram_tensor("bias_v", [1, NV], F32, kind="ExternalInput").ap(),
    ]
    outs = [
        nc.dram_tensor("logits_o", [1, NV], F32, kind="ExternalOutput").ap(),
        nc.dram_tensor("stats_o", [2, 16], F32, kind="ExternalOutput").ap(),
        nc.dram_tensor("h_o", [128, 16], F32R, kind="ExternalOutput").ap(),
    ]
    nrep = int(os.environ.get("KNREP", "1"))  # timing: unroll body N times
    with tile.TileContext(nc) as tc:
        for _ in range(nrep):
            with ExitStack() as ctx:
                _build_body(ctx, tc, ins, outs)
    nc.compile()
    _CACHED_NC = nc
    return nc


def kernel(input_tok, hidden, encoder_hiddens, emb, W_ih, W_hh, b_ih, b_hh, W_out, b_out):
    global LAST_RESULTS
    tok = int(np.asarray(input_tok).reshape(-1)[0])
    x = np.ascontiguousarray(np.asarray(emb)[tok], dtype=np.float32)
    hid = np.asarray(hidden, np.float32)
    enc = np.asarray(encoder_hiddens, np.float32)
    W_ih = np.asarray(W_ih, np.float32)
    W_hh = np.asarray(W_hh, np.float32)
    b_ih = np.asarray(b_ih, np.float32)
    b_hh = np.asarray(b_hh, np.float32)
    W_out = np.asarray(W_out, np.float32)
    b_out = np.asarray(b_out, np.float32)

    # ucat_d = [x; h0_d]; xh[p, c, d] = ucat_d[128*c + p]
    ucat = np.stack(
        [np.concatenate([x, hid[0, 0]]), np.concatenate([x, hid[1, 0]])], axis=-1
    )  # (2048, 2)
    xh = np.ascontiguousarray(ucat.reshape(16, 128, 2).transpose(1, 0, 2))

    wcat_t = np.ascontiguousarray(
        np.concatenate([W_ih, W_hh], axis=2).transpose(0, 2, 1)
    )  # (2, 2048, 1024)
    bias_h = np.ascontiguousarray((b_ih + b_hh).reshape(1, 2 * H))
    enc_t = np.ascontiguousarray(enc.T)

    import ml_dtypes

    common = {
        "xh": xh,
        "wcat_t": wcat_t,
        "bias_h": bias_h,
        "enc_t": enc_t,
        "enc_n": np.ascontiguousarray(enc).astype(ml_dtypes.bfloat16),
        "id2": np.eye(2, dtype=np.float32),
    }
    in_maps = []
    for c in range(NCORES):
        lo = c * NV
        hi = min(lo + NV, V)
        wt = np.zeros((K, NV), ml_dtypes.bfloat16)
        wt[:, : hi - lo] = W_out[lo:hi].T.astype(ml_dtypes.bfloat16)
        bv = np.full((1, NV), NEG, np.float32)
        bv[0, : hi - lo] = b_out[lo:hi]
        in_maps.append({**common, "w_out_t": wt, "bias_v": bv})

    nc = _get_program()
    res = run_bass_kernel_spmd(nc, in_maps, core_ids=list(range(NCORES)))
    LAST_RESULTS = res

    logits = np.concatenate([res.results[c]["logits_o"][0] for c in range(NCORES)])[:V]
    mv = np.stack([res.results[c]["stats_o"][0, :NSLABS] for c in range(NCORES)])
    sv = np.stack([res.results[c]["stats_o"][1, :NSLABS] for c in range(NCORES)])
    M = float(mv.max())
    lse = M + np.log(np.sum(sv * np.exp(mv - M)))
    out = (logits - lse).astype(np.float32)[None, :]

    h_np = res.results[0]["h_o"]  # (128, 16): [p, d*8 + c] = h[d, 128*c + p]
    h = np.stack(
        [h_np[:, d * 8 : d * 8 + 8].T.reshape(H) for d in range(2)]
    ).astype(np.float32)[:, None, :]
    return out, h


# revision 69
# speedup vs baseline: 1.0005x; 1.0005x over previous
"""Trainium2 Bass kernel for a single-step decoder BiRNN with attention + vocab projection.

Computation (see reference):
  x = emb[tok]                                  (1024,)
  h_d = tanh(W_ih[d] x + b_ih[d] + W_hh[d] h0_d + b_hh[d])        d in {0,1}
  energy = h @ enc^T -> softmax over L -> attn_out = attn @ enc   (2, 1024)
  u = concat(attn_out0, h0, attn_out1, h1)      (4096,)
  logits = W_out @ u + b_out                    (50257,)
  out = log_softmax(logits), also return h

Sharding: W_out rows (vocab) split across 8 cores (6400 padded rows each,
pre-transposed on the host so DMA lines are contiguous).  The tiny RNN cell and
attention are replicated on every core.  Each core returns its logits shard plus
per-slab (max, sum-exp) partials; the host combines those into the global
log-softmax normalizer.
"""

import os
import sys
from contextlib import ExitStack

import numpy as np

if "/opt/trn_rl_repo" not in sys.path:
    sys.path.insert(0, "/opt/trn_rl_repo")

import concourse.bass as bass
import concourse.tile as tile
from concourse import bacc, mybir
from concourse.bass_utils import run_bass_kernel_spmd

H = 1024
L = 2048
V = 50257
NCORES = 8
NV = 6284            # per-core padded vocab rows (8 * 6284 = 50272 >= V)
K = 4 * H            # contraction dim of the output projection
NEG = -1e9           # bias for padded vocab rows (exp underflows to exactly 0)

F32 = mybir.dt.float32
F32R = mybir.dt.float32r   # full-rate PE at N>=256, same bits as fp32
BF16 = mybir.dt.bfloat16
AX = mybir.AxisListType
AF = mybir.ActivationFunctionType
ALU = mybir.AluOpType

# (voff, vlen, nslabs): sequential passes over the vocab shard so the live
# [1, 512] PSUM accumulators never exceed the 8 banks.
PASSES = [(0, 2048, 4), (2048, 2048, 4), (4096, 2048, 4), (6144, 140, 1)]
NSLABS = 13

# Results of the last device run (test harness reads exec_time_ns from here).
LAST_RESULTS = None


def _build_body(ctx: ExitStack, tc: tile.TileContext, ins, outs):
    stage = int(os.environ.get("KSTAGE", "4"))  # debug bisect knob
    nc = tc.nc
    xh, wcat_t, bias_h, enc_t, enc_n, id2, w_out_t, bias_v = ins
    logits_o, stats_o, h_o = outs

    const = ctx.enter_context(tc.tile_pool(name="const", bufs=1))
    small = ctx.enter_context(tc.tile_pool(name="small", bufs=2))
    wcat_pool = ctx.enter_context(tc.tile_pool(name="wcat", bufs=2))
    w_pool = ctx.enter_context(tc.tile_pool(name="w", bufs=6))
    scr_pool = ctx.enter_context(tc.tile_pool(name="scr", bufs=2))
    encn_pool = ctx.enter_context(tc.tile_pool(name="encn", bufs=2))
    bvs_pool = ctx.enter_context(tc.tile_pool(name="bvs", bufs=2))
    lgsb_pool = ctx.enter_context(tc.tile_pool(name="lgsb", bufs=2))
    ps_pool = ctx.enter_context(tc.tile_pool(name="ps", bufs=4, space="PSUM"))

    # ---------------- small resident inputs ----------------
    xh_sb = const.tile([128, 16, 2], F32R)
    nc.sync.dma_start(xh_sb[:], xh[:])
    biash_sb = const.tile([1, 2 * H], F32)
    nc.sync.dma_start(biash_sb[:], bias_h[:])
    one_sb = const.tile([1, 1], F32)
    nc.vector.memset(one_sb[:], 1.0)
    id2_sb = const.tile([2, 2], F32)
    nc.sync.dma_start(id2_sb[:], id2[:])

    # u vector as 32 column chunks: col j = 16*d + 8*s + c  (s=0: attn_out, s=1: h)
    u_sb = const.tile([128, 32], F32R)
    u4 = u_sb.rearrange("p (d s c) -> p d s c", d=2, s=2)

    # ---------------- RNN cell: h_d = tanh(Wcat_d @ ucat_d + b_d) ----------------
    h_pre = [[ps_pool.tile([1, 512], F32, name=f"hpre_{d}_{hs}", tag="pbank", bufs=4) for hs in range(2)] for d in range(2)]
    for d in range(2):
        wc_r = wcat_t.rearrange("d (c p) h -> p d c h", p=128)
        for ci in range(8):  # two 128-row k-chunks per 1MB DMA
            wt = wcat_pool.tile([128, 2, H], F32R, tag="wcat")
            nc.sync.dma_start(wt[:], wc_r[:, d, 2 * ci : 2 * ci + 2, :])
            for cc in range(2):
                c = 2 * ci + cc
                for hs in range(2):
                    nc.tensor.matmul(
                        h_pre[d][hs][0:1, :],
                        lhsT=xh_sb[:, c : c + 1, d],
                        rhs=wt[:, cc, 512 * hs : 512 * hs + 512],
                        start=(c == 0),
                        stop=(c == 15),
                    )

    # enc^T resident: [p, hc, l] = enc_t[128*hc + p, l].  Emitted after the
    # recurrent-weight stream so wcat keeps DMA priority (the whole attention
    # chain is gated on it); split into 4 transfers for queue balance.
    enc_sb = const.tile([128, 8, L], F32R)
    enc_r = enc_t.rearrange("(hc p) l -> p hc l", p=128)
    for g in range(4):
        nc.sync.dma_start(enc_sb[:, 2 * g : 2 * g + 2, :], enc_r[:, 2 * g : 2 * g + 2, :])

    h_sb = const.tile([1, 2 * H], F32)
    for d in range(2):
        for hs in range(2):
            lo = d * H + hs * 512
            tmp = small.tile([1, 512], F32, tag="htmp")
            nc.vector.tensor_add(tmp[:], h_pre[d][hs][0:1, :], biash_sb[0:1, lo : lo + 512])
            nc.scalar.activation(h_sb[0:1, lo : lo + 512], tmp[:], AF.Tanh)

    # transpose h into u columns via K=1 outer product with ones
    for d in range(2):
        tp = ps_pool.tile([128, 8], F32, name=f"tp_{d}", tag="pbank", bufs=4)
        for c in range(8):
            nc.tensor.matmul(
                tp[:, c : c + 1],
                lhsT=h_sb[0:1, (d * 8 + c) * 128 : (d * 8 + c) * 128 + 128],
                rhs=one_sb[0:1, 0:1],
                start=True,
                stop=True,
            )
        nc.vector.tensor_copy(u4[:, d, 1, :], tp[:])

    nc.sync.dma_start(h_o.rearrange("p (d c) -> p d c", d=2), u4[:, :, 1, :])
    if stage < 2:
        return

    # bf16 u for the projection; h columns are ready now, attn columns later.
    # Per-direction copies so part 1 can start on d=0's chunks while d=1's
    # recurrent weights are still streaming.
    u_bf = const.tile([128, 32], BF16)
    ub4 = u_bf.rearrange("p (d s c) -> p d s c", d=2, s=2)
    for d in range(2):
        nc.vector.tensor_copy(ub4[:, d, 1, :], u4[:, d, 1, :])

    # partial logits accumulator (h-chunk contribution, computed pre-attention
    # so the W_out stream never stalls on the attention tail)
    part_sb = const.tile([1, NV], F32)
    w_r = w_out_t.rearrange("(c p) v -> p c v", p=128)
    mv = const.tile([1, 16], F32)
    negmv = const.tile([1, 16], F32)
    sv = const.tile([1, 16], F32)
    nc.vector.memset(mv[:], 0.0)
    nc.vector.memset(negmv[:], 0.0)
    nc.vector.memset(sv[:], 0.0)

    def projection_part(pair_starts, store_partial):
        idx = 0
        tagc = "p" if store_partial else "f"
        for voff, vlen, nslabs in PASSES:
            psums = [
                ps_pool.tile(
                    [1, 512], F32, name=f"lg{tagc}_{voff}_{s}", tag="pbank", bufs=4
                )
                for s in range(nslabs)
            ]
            for pi, c0 in enumerate(pair_starts):
                wt = w_pool.tile([128, 2, 2048], BF16, tag="w")
                nc.sync.dma_start(
                    wt[:, :, :vlen], w_r[:, c0 : c0 + 2, voff : voff + vlen]
                )
                for cc in range(2):
                    for s in range(nslabs):
                        so = 512 * s
                        sl = min(512, vlen - so)
                        nc.tensor.matmul(
                            psums[s][0:1, :sl],
                            lhsT=u_bf[:, c0 + cc : c0 + cc + 1],
                            rhs=wt[:, cc, so : so + sl],
                            start=(pi == 0 and cc == 0),
                            stop=(pi == len(pair_starts) - 1 and cc == 1),
                        )
            for s in range(nslabs):
                so = 512 * s
                sl = min(512, vlen - so)
                g0 = voff + so
                if store_partial:
                    nc.vector.tensor_copy(part_sb[0:1, g0 : g0 + sl], psums[s][0:1, :sl])
                else:
                    bvt = bvs_pool.tile([1, 512], F32, name=f"bv_{g0}", tag="bvs")
                    nc.sync.dma_start(bvt[0:1, :sl], bias_v[0:1, g0 : g0 + sl])
                    lgt = lgsb_pool.tile([1, 512], F32, name=f"lgsb_{g0}", tag="lgsb")
                    nc.vector.tensor_add(
                        lgt[0:1, :sl], psums[s][0:1, :sl], part_sb[0:1, g0 : g0 + sl]
                    )
                    nc.vector.tensor_add(lgt[0:1, :sl], lgt[0:1, :sl], bvt[0:1, :sl])
                    nc.vector.reduce_max(
                        mv[0:1, idx : idx + 1], lgt[0:1, :sl], axis=AX.X
                    )
                    nc.scalar.mul(
                        negmv[0:1, idx : idx + 1], mv[0:1, idx : idx + 1], -1.0
                    )
                    scr = scr_pool.tile([1, 512], F32, tag="exps")
                    nc.scalar.activation(
                        scr[0:1, :sl],
                        lgt[0:1, :sl],
                        AF.Exp,
                        bias=negmv[0:1, idx : idx + 1],
                        accum_out=sv[0:1, idx : idx + 1],
                    )
                    nc.sync.dma_start(logits_o[0:1, g0 : g0 + sl], lgt[0:1, :sl])
                idx += 1

    # part 1: h-chunk columns of W_out (u columns 8..15 and 24..31)
    projection_part([8, 10, 12, 14, 24, 26, 28, 30], store_partial=True)

    # ---------------- attention ----------------
    # energy[d, l] = sum_h h[d, h] * enc[l, h] for both directions at once
    # (fp32 for accuracy: the softmax is sensitive to energy error).
    eps2 = ps_pool.tile([2, L], F32, name="eps2", tag="pwide", bufs=1)
    for hc in range(8):
        for lt in range(4):
            nc.tensor.matmul(
                eps2[:, 512 * lt : 512 * lt + 512],
                lhsT=u4[:, :, 1, hc],
                rhs=enc_sb[:, hc, 512 * lt : 512 * lt + 512],
                start=(hc == 0),
                stop=(hc == 7),
            )

    # joint softmax over the free (L) axis for both directions
    st = const.tile([2, 4], F32)  # cols: m, -m, s, 1/s
    attn2 = const.tile([2, L], F32)
    nc.vector.reduce_max(st[:, 0:1], eps2[:], axis=AX.X)
    nc.scalar.mul(st[:, 1:2], st[:, 0:1], -1.0)
    nc.scalar.activation(
        attn2[:], eps2[:], AF.Exp, bias=st[:, 1:2], accum_out=st[:, 2:3]
    )
    nc.vector.reciprocal(st[:, 3:4], st[:, 2:3])
    nc.vector.tensor_scalar_mul(attn2[:], attn2[:], st[:, 3:4])

    if stage < 3:
        return
    # ---------------- attn_out = attn @ enc (PE path, bf16) ----------------
    # transpose attn into l-partition chunks via identity: attnT_sb[p, lc, d]
    attnT_sb = const.tile([128, 16, 2], BF16)
    for g in range(2):
        tpa = ps_pool.tile([128, 16], F32, name=f"tpa_{g}", tag="pwide", bufs=1)
        for j in range(8):
            lc = g * 8 + j
            nc.tensor.matmul(
                tpa[:, 2 * j : 2 * j + 2],
                lhsT=attn2[:, 128 * lc : 128 * lc + 128],
                rhs=id2_sb[:],
                start=True,
                stop=True,
            )
        nc.vector.tensor_copy(
            attnT_sb[:, 8 * g : 8 * g + 8, :], tpa[:].rearrange("p (c d) -> p c d", d=2)
        )

    # attn_out[d, :] = sum_l attn[d, l] * enc_n[l, :] (bf16, both directions)
    ao_all = ps_pool.tile([2, 2, 512], F32, name="ao_all", tag="pwide", bufs=1)
    ao_ps = [ao_all[:, hs, :] for hs in range(2)]
    en_r = enc_n.rearrange("(c p) h -> p c h", p=128)
    for lci in range(8):
        et = encn_pool.tile([128, 2, H], BF16, tag="encn")
        nc.sync.dma_start(et[:], en_r[:, 2 * lci : 2 * lci + 2, :])
        for j in range(2):
            lc = 2 * lci + j
            for hs in range(2):
                nc.tensor.matmul(
                    ao_ps[hs],
                    lhsT=attnT_sb[:, lc, :],
                    rhs=et[:, j, 512 * hs : 512 * hs + 512],
                    start=(lc == 0),
                    stop=(lc == 15),
                )

    # transpose attn_out into u columns via identity
    ao_sb = const.tile([2, H], F32)
    for hs in range(2):
        nc.vector.tensor_copy(ao_sb[:, 512 * hs : 512 * hs + 512], ao_ps[hs])
    tpo = ps_pool.tile([128, 16], F32, name="tpo", tag="pwide", bufs=1)
    for c in range(8):
        nc.tensor.matmul(
            tpo[:, 2 * c : 2 * c + 2],
            lhsT=ao_sb[:, 128 * c : 128 * c + 128],
            rhs=id2_sb[:],
            start=True,
            stop=True,
        )
    nc.vector.tensor_copy(u4[:, :, 0, :], tpo[:].rearrange("p (c d) -> p d c", d=2))
    nc.vector.tensor_copy(ub4[:, :, 0, :], tpo[:].rearrange("p (c d) -> p d c", d=2))

    if stage < 4:
        return
    # part 2: attn-chunk columns of W_out (u columns 0..7 and 16..23)
    projection_part([0, 2, 4, 6, 16, 18, 20, 22], store_partial=False)

    # ---------------- outputs ----------------
    nc.sync.dma_start(stats_o[0:1, :], mv[:])
    nc.sync.dma_start(stats_o[1:2, :], sv[:])


_CACHED_NC = None


def _get_program():
    global _CACHED_NC
    if _CACHED_NC is not None:
        return _CACHED_NC
    nc = bacc.Bacc(
        "TRN2",
        target_bir_lowering=False,
        debug=False,
        enable_asserts=False,
        num_devices=NCORES,
    )
    ins = [
        nc.dram_tensor("xh", [128, 16, 2], F32R, kind="ExternalInput").ap(),
        nc.dram_tensor("wcat_t", [2, 2 * H, H], F32R, kind="ExternalInput").ap(),
        nc.dram_tensor("bias_h", [1, 2 * H], F32, kind="ExternalInput").ap(),
        nc.dram_tensor("enc_t", [H, L], F32R, kind="ExternalInput").ap(),
        nc.dram_tensor("enc_n", [L, H], BF16, kind="ExternalInput").ap(),
        nc.dram_tensor("id2", [2, 2], F32, kind="ExternalInput").ap(),
        nc.dram_tensor("w_out_t", [K, NV], BF16, kind="ExternalInput").ap(),
        nc.dram_tensor("bias_v", [1, NV], F32, kind="ExternalInput").ap(),
    ]
    outs = [
        nc.dram_tensor("logits_o", [1, NV], F32, kind="ExternalOutput").ap(),
        nc.dram_tensor("stats_o", [2, 16], F32, kind="ExternalOutput").ap(),
        nc.dram_tensor("h_o", [128, 16], F32R, kind="ExternalOutput").ap(),
    ]
    nrep = int(os.environ.get("KNREP", "1"))  # timing: unroll body N times
    with tile.TileContext(nc) as tc:
        for _ in range(nrep):
            with ExitStack() as ctx:
                _build_body(ctx, tc, ins, outs)
    nc.compile()
    _CACHED_NC = nc
    return nc


def kernel(input_tok, hidden, encoder_hiddens, emb, W_ih, W_hh, b_ih, b_hh, W_out, b_out):
    global LAST_RESULTS
    tok = int(np.asarray(input_tok).reshape(-1)[0])
    x = np.ascontiguousarray(np.asarray(emb)[tok], dtype=np.float32)
    hid = np.asarray(hidden, np.float32)
    enc = np.asarray(encoder_hiddens, np.float32)
    W_ih = np.asarray(W_ih, np.float32)
    W_hh = np.asarray(W_hh, np.float32)
    b_ih = np.asarray(b_ih, np.float32)
    b_hh = np.asarray(b_hh, np.float32)
    W_out = np.asarray(W_out, np.float32)
    b_out = np.asarray(b_out, np.float32)

    # ucat_d = [x; h0_d]; xh[p, c, d] = ucat_d[128*c + p]
    ucat = np.stack(
        [np.concatenate([x, hid[0, 0]]), np.concatenate([x, hid[1, 0]])], axis=-1
    )  # (2048, 2)
    xh = np.ascontiguousarray(ucat.reshape(16, 128, 2).transpose(1, 0, 2))

    wcat_t = np.ascontiguousarray(
        np.concatenate([W_ih, W_hh], axis=2).transpose(0, 2, 1)
    )  # (2, 2048, 1024)
    bias_h = np.ascontiguousarray((b_ih + b_hh).reshape(1, 2 * H))
    enc_t = np.ascontiguousarray(enc.T)

    import ml_dtypes

    common = {
        "xh": xh,
        "wcat_t": wcat_t,
        "bias_h": bias_h,
        "enc_t": enc_t,
        "enc_n": np.ascontiguousarray(enc).astype(ml_dtypes.bfloat16),
        "id2": np.eye(2, dtype=np.float32),
    }
    in_maps = []
    for c in range(NCORES):
        lo = c * NV
        hi = min(lo + NV, V)
        wt = np.zeros((K, NV), ml_dtypes.bfloat16)
        wt[:, : hi - lo] = W_out[lo:hi].T.astype(ml_dtypes.bfloat16)
        bv = np.full((1, NV), NEG, np.float32)
        bv[0, : hi - lo] = b_out[lo:hi]
        in_maps.append({**common, "w_out_t": wt, "bias_v": bv})

    nc = _get_program()
    res = run_bass_kernel_spmd(nc, in_maps, core_ids=list(range(NCORES)))
    LAST_RESULTS = res

    logits = np.concatenate([res.results[c]["logits_o"][0] for c in range(NCORES)])[:V]
    mv = np.stack([res.results[c]["stats_o"][0, :NSLABS] for c in range(NCORES)])
    sv = np.stack([res.results[c]["stats_o"][1, :NSLABS] for c in range(NCORES)])
    M = float(mv.max())
    lse = M + np.log(np.sum(sv * np.exp(mv - M)))
    out = (logits - lse).astype(np.float32)[None, :]

    h_np = res.results[0]["h_o"]  # (128, 16): [p, d*8 + c] = h[d, 128*c + p]
    h = np.stack(
        [h_np[:, d * 8 : d * 8 + 8].T.reshape(H) for d in range(2)]
    ).astype(np.float32)[:, None, :]
    return out, h
